# revision 1
# baseline (speedup 1.0000x reference)
"""Trainium2 Bass kernel for nn_Attention_48498770706573.

Fused QKV-projection + masked softmax attention, sharded over 8 NeuronCores:
data-parallel over batch (B=2), tensor-parallel over heads (16 -> 4 per
core). Each core computes its (batch, 4-head) shard end to end; the host
only slices/transposes inputs (layout only, no arithmetic) and concatenates
the disjoint output shards.

Per-core dataflow (all "transposed-land"):
  qT/kT/vT [D, N] fp32 DRAM declared float32r, converted to bf16 on-chip,
  projections (bf16 matmuls, fp32 PSUM) -> qtT/ktT [256, N] and vt [N, 256],
  scores S^T[nk, nq] as bf16 matmuls with two heads row-packed (K=64 each),
  exp on ACT straight out of PSUM -> bf16 (1/32 scale folded in),
  bool mask cast u8->bf16 via SWDGE cast-DMA, then one broadcast DVE
  multiply (2x bf16 mode) per score group,
  PV with p^T (bf16) as moving operand, two heads col-packed per PSUM bank,
  row-sums via ones-column matmuls, 4 heads col-packed into one PSUM bank,
  V-bias as a rank-1 (bv x rowsum) matmul and normalization via a rank-1
  broadcast of 1/(rowsum+1e-6), then one DVE multiply per output tile.
  The PV/rowsum stream lags the score stream by 3 k-tiles (software
  pipeline) and each chunk's rowsum->reciprocal tail is deferred into the
  next chunk's first score groups.
"""

import os

import numpy as np

import concourse.bacc as bacc
import concourse.hw_specs as _hw_specs
import concourse.mybir as mybir
import concourse.tile as tile
from concourse.bass_utils import run_bass_kernel_spmd

# The kernel uses both Exp and Ln; steer both into the combined
# "natural_log_exp_and_others" ACT table set so chunk tails don't thrash
# the table RAM (~2.7us per switch). Dict order (= act_func_set_id) kept.
_orig_get_act_tables = _hw_specs.get_activation_tables


def _patched_get_act_tables(module_arch):
    exp = mybir.ActivationFunctionType.Exp
    ln = mybir.ActivationFunctionType.Ln
    out = {}
    for name, funcs in _orig_get_act_tables(module_arch).items():
        f = set(funcs)
        if name != "natural_log_exp_and_others":
            f.discard(exp)
            f.discard(ln)
        out[name] = f
    return out


_hw_specs.get_activation_tables = _patched_get_act_tables
bacc.get_activation_tables = _patched_get_act_tables

B, NQ, NK, D, H = 2, 2048, 2048, 1024, 16
DH = D // H  # 64
N_CORES = 8
HPC = H // (N_CORES // B)  # heads per core = 4
JW = HPC * DH  # per-core projection width = 256
NKT = NK // 128  # 16 nk tiles
NCH = 4  # nq chunks
CHW = NQ // NCH  # 512
DT = 8  # contraction d-tiles

f32 = mybir.dt.float32
f32r = mybir.dt.float32r
bf16 = mybir.dt.bfloat16
u8 = mybir.dt.uint8


def _build():
    nc = bacc.Bacc(
        "TRN2", target_bir_lowering=False, debug=False, num_devices=N_CORES
    )

    qT = nc.dram_tensor("qT", [D, NQ], f32r, kind="ExternalInput")
    kT = nc.dram_tensor("kT", [D, NK], f32r, kind="ExternalInput")
    vT = nc.dram_tensor("vT", [D, NK], f32r, kind="ExternalInput")
    maskT = nc.dram_tensor("maskT", [NK, NQ], u8, kind="ExternalInput")
    wqT = nc.dram_tensor("wqT", [D, JW], f32r, kind="ExternalInput")
    wkT = nc.dram_tensor("wkT", [D, JW], f32r, kind="ExternalInput")
    wvT = nc.dram_tensor("wvT", [D, JW], f32r, kind="ExternalInput")
    bqd = nc.dram_tensor("bq", [2, 128], f32, kind="ExternalInput")
    bkd = nc.dram_tensor("bk", [2, 128], f32, kind="ExternalInput")
    # bv2[64p + 0, 0:64] = bv[128p + dh], bv2[64p + 32, 64:128] = bv[...]
    bvd = nc.dram_tensor("bv2", [128, 128], f32r, kind="ExternalInput")
    # ones2[64p, 0:64] = 1, ones2[64p + 32, 64:128] = 1
    onesd = nc.dram_tensor("ones2", [128, 128], f32r, kind="ExternalInput")
    onespd = nc.dram_tensor("onesp", [128, 32], bf16, kind="ExternalInput")
    o = nc.dram_tensor("o", [2 * 128, NQ], f32, kind="ExternalOutput")

    with tile.TileContext(nc) as tc:
        with (
            tc.tile_pool(name="consts", bufs=1) as consts,
            tc.tile_pool(name="wtmp", bufs=1) as wtmp,
            tc.tile_pool(name="stage", bufs=12) as stage,
            tc.tile_pool(name="vbfp", bufs=8) as vbfp,
            tc.tile_pool(name="xbfp", bufs=10) as xbfp,
            tc.tile_pool(name="qpool", bufs=12) as qpool,
            tc.tile_pool(name="m8pool", bufs=16) as m8pool,
            tc.tile_pool(name="mbpool", bufs=6) as mbpool,
            tc.tile_pool(name="projout", bufs=1) as projout,
            tc.tile_pool(name="ppool", bufs=9) as ppool,
            tc.tile_pool(name="rspool", bufs=2) as rspool,
            tc.tile_pool(name="outsb", bufs=4) as outsb,
            tc.tile_pool(name="sps", bufs=2, space="PSUM") as sps,
            tc.tile_pool(name="pvps", bufs=2, space="PSUM") as pvps,
            tc.tile_pool(name="rsps", bufs=2, space="PSUM") as rsps,
        ):
            # ---- constants ----
            w_sb = {}

            def dma_w(name, dram):
                t = wtmp.tile([128, DT, JW], f32r, tag=f"wt{name}", name="wt")
                for d in range(DT):
                    nc.sync.dma_start(t[:, d], dram[d * 128 : (d + 1) * 128, :])
                return t

            def conv_w(name, t):
                wb = consts.tile([128, DT, JW], bf16, tag=f"w{name}", name="w")
                for d in range(DT):
                    nc.vector.tensor_copy(wb[:, d], t[:, d])
                w_sb[name] = wb

            # ---- decoupled input DMAs (emitted in priority order) ----
            def dma_x_chunk(src, ch, tiles=None, pool=None):
                pool = pool or stage
                tiles = {} if tiles is None else tiles
                for d in range(DT):
                    x = pool.tile([128, CHW], f32r, tag="xc", name="x")
                    nc.sync.dma_start(
                        x,
                        src[d * 128 : (d + 1) * 128, ch * CHW : (ch + 1) * CHW],
                    )
                    tiles[(d, ch)] = x
                return tiles

            bq_sb = consts.tile([128, 2], f32, tag="bq")
            bk_sb = consts.tile([128, 2], f32, tag="bk")
            for m in range(2):
                nc.sync.dma_start(
                    bq_sb[:, m : m + 1],
                    bqd[m : m + 1, :].rearrange("a b -> b a"),
                )
                nc.sync.dma_start(
                    bk_sb[:, m : m + 1],
                    bkd[m : m + 1, :].rearrange("a b -> b a"),
                )
            bv_sb = consts.tile([128, 128], f32r, tag="bv")
            nc.sync.dma_start(bv_sb, bvd[:])
            ones_sb = consts.tile([128, 128], f32r, tag="ones")
            nc.sync.dma_start(ones_sb, onesd[:])
            onesp_sb = consts.tile([128, 32], bf16, tag="onesp")
            nc.sync.dma_start(onesp_sb, onespd[:])
            wtk = dma_w("k", wkT)
            conv_w("k", wtk)
            k_tiles = {}
            for d in range(DT):
                for ch in range(NCH):
                    x = stage.tile([128, CHW], f32r, tag="xc", name="x")
                    nc.sync.dma_start(
                        x, kT[d * 128 : (d + 1) * 128, ch * CHW : (ch + 1) * CHW]
                    )
                    k_tiles[(d, ch)] = x
            wtq = dma_w("q", wqT)
            wtv = dma_w("v", wvT)
            q_tiles = dma_x_chunk(qT, 0)
            m8 = []
            for t in range(NKT):
                mt8 = m8pool.tile([128, NQ], u8, tag="m8", name="m8")
                nc.sync.dma_start(mt8, maskT[t * 128 : (t + 1) * 128, :])
                m8.append(mt8)
            v_tiles = {}
            for ch in range(NCH):
                dma_x_chunk(vT, ch, v_tiles)
            q_later = {}
            for ch in range(1, NCH):
                dma_x_chunk(qT, ch, q_later, pool=qpool)

            # ---- projections ----
            qtT = projout.tile([128, 2, NQ], bf16, tag="qtT")
            ktT = projout.tile([128, 2, NK], bf16, tag="ktT")
            vt = projout.tile([128, NKT, JW], bf16, tag="vt")

            def proj_qk_full(name, tiles, dst, bias):
                """All 4 chunks; m0 accumulates into two 2-bank sps tiles,
                m1 into four psum singles."""
                ps0t = [
                    sps.tile([128, 2 * CHW], f32, tag="s", name=f"ps0{i}")
                    for i in range(2)
                ]
                ps0 = [
                    ps0t[0][:, 0:CHW],
                    ps0t[0][:, CHW:],
                    ps0t[1][:, 0:CHW],
                    ps0t[1][:, CHW:],
                ]
                ps1 = [
                    pvps.tile([128, CHW], f32, tag="pvpst", name=f"psa{i}")
                    for i in range(2)
                ] + [
                    rsps.tile([128, CHW], f32, tag="rspst", name=f"psb{i}")
                    for i in range(2)
                ]
                for d in range(DT):
                    xb = {}
                    for ch in range(NCH):
                        xb[ch] = xbfp.tile(
                            [128, CHW], bf16, tag="xb", name="xb"
                        )
                        nc.vector.tensor_copy(xb[ch], tiles[(d, ch)])
                    for ch in range(NCH):
                        nc.tensor.matmul(
                            ps0[ch],
                            w_sb[name][:, d, 0:128],
                            xb[ch],
                            start=(d == 0),
                            stop=(d == DT - 1),
                        )
                    for ch in range(NCH):
                        nc.tensor.matmul(
                            ps1[ch],
                            w_sb[name][:, d, 128:256],
                            xb[ch],
                            start=(d == 0),
                            stop=(d == DT - 1),
                        )
                for ch2 in range(2):
                    nc.vector.tensor_scalar_add(
                        dst[:, 0, ch2 * 2 * CHW : (ch2 + 1) * 2 * CHW],
                        ps0t[ch2],
                        bias[:, 0:1],
                    )
                for ch in range(NCH):
                    nc.vector.tensor_scalar_add(
                        dst[:, 1, ch * CHW : (ch + 1) * CHW],
                        ps1[ch],
                        bias[:, 1:2],
                    )

            def proj_qk_chunk(name, tiles, ch, dst, bias, cast_dma=False):
                xb = {}
                for d in range(DT):
                    xb[d] = xbfp.tile([128, CHW], bf16, tag="xb", name="xb")
                    if cast_dma:
                        nc.gpsimd.dma_start(xb[d], tiles[(d, ch)])
                    else:
                        nc.vector.tensor_copy(xb[d], tiles[(d, ch)])
                for m in range(2):
                    ps = rsps.tile([128, CHW], f32, tag="rspst", name="pps")
                    for d in range(DT):
                        nc.tensor.matmul(
                            ps,
                            w_sb[name][:, d, m * 128 : (m + 1) * 128],
                            xb[d],
                            start=(d == 0),
                            stop=(d == DT - 1),
                        )
                    nc.vector.tensor_scalar_add(
                        dst[:, m, ch * CHW : (ch + 1) * CHW],
                        ps,
                        bias[:, m : m + 1],
                    )

            proj_qk_full("k", k_tiles, ktT, bk_sb)
            conv_w("q", wtq)
            conv_w("v", wtv)
            wv_bf = w_sb["v"]
            proj_qk_chunk("q", q_tiles, 0, qtT, bq_sb)

            def proj_v():
                # bf16 x-tiles so the weight loads pipeline with the matmuls
                vbf = {}
                for n in range(NKT):
                    ch, nn_ = divmod(n, 4)
                    ps = pvps.tile([128, JW], f32, tag="pvpst", name="vps")
                    for d in range(DT):
                        if (d, ch) not in vbf:
                            xb = vbfp.tile(
                                [128, CHW], bf16, tag="vb", name="vb"
                            )
                            nc.vector.tensor_copy(xb, v_tiles[(d, ch)])
                            vbf[(d, ch)] = xb
                        nc.tensor.matmul(
                            ps,
                            vbf[(d, ch)][:, nn_ * 128 : (nn_ + 1) * 128],
                            wv_bf[:, d, :],
                            start=(d == 0),
                            stop=(d == DT - 1),
                        )
                    nc.vector.tensor_copy(vt[:, n, :], ps)

            # ---- attention ----
            def scores_group(pair, t, cs, p_tiles):
                sp = sps.tile([128, 2 * CHW], f32, tag="s", name="sp")
                for hh in range(2):
                    nc.tensor.matmul(
                        sp[:, hh * CHW : (hh + 1) * CHW],
                        ktT[
                            64 * hh : 64 * (hh + 1),
                            pair,
                            t * 128 : (t + 1) * 128,
                        ],
                        qtT[64 * hh : 64 * (hh + 1), pair, cs],
                        start=True,
                        stop=True,
                    )
                p = ppool.tile([128, 2 * CHW], bf16, tag="p", name="p")
                nc.scalar.activation(
                    out=p,
                    in_=sp,
                    func=mybir.ActivationFunctionType.Exp,
                    scale=1.0 / 32.0,
                )
                if pair == 0:
                    mb = mbpool.tile([128, CHW], bf16, tag="mb", name="mb")
                    # SWDGE cast DMA u8 -> bf16 (frees GpSimd compute)
                    nc.gpsimd.dma_start(mb, m8[t][:, cs])
                    p_tiles[("mb", t)] = mb
                else:
                    mb = p_tiles[("mb", t)]
                p3 = p.rearrange("p (h c) -> p h c", h=2)
                nc.vector.tensor_mul(
                    p3,
                    p3,
                    mb.rearrange("p (a c) -> p a c", a=1).to_broadcast(
                        (128, 2, CHW)
                    ),
                )
                p_tiles[(pair, t)] = p

            def pv_t(t, p_tiles, pv_ps, rs_ps):
                st, sp_ = t == 0, t == NKT - 1
                for pair in range(2):
                    p = p_tiles[(pair, t)]
                    for hh in range(2):
                        nc.tensor.matmul(
                            pv_ps[pair][64 * hh : 64 * (hh + 1), :],
                            vt[
                                :,
                                t,
                                128 * pair + 64 * hh : 128 * pair
                                + 64 * (hh + 1),
                            ],
                            p[:, hh * CHW : (hh + 1) * CHW],
                            start=st,
                            stop=sp_,
                            tile_position=(0, 64 * hh),
                        )
                for pair in range(2):
                    p = p_tiles[(pair, t)]
                    for hh in range(2):
                        hg = 2 * pair + hh
                        nc.tensor.matmul(
                            rs_ps[32 * hg : 32 * hg + 32, :],
                            onesp_sb[:, 0:32],
                            p[:, hh * CHW : (hh + 1) * CHW],
                            start=st,
                            stop=sp_,
                            tile_position=(0, 32 * hg),
                        )

            def chunk_tail(cs, pv_ps, rs_ps):
                # rowsum -> +eps -> reciprocal (all 128 rows valid: the M=32
                # rowsum matmuls wrote 32 identical rows per head)
                rs_sb = rspool.tile([128, CHW], f32r, tag="rssb", name="rssb")
                nc.vector.tensor_scalar_add(rs_sb, rs_ps, 1e-6)
                # 1/rs via ACT exp(-ln(rs)): ~1.5us vs the 4us DVE
                # reciprocal, and off the DVE queue; eps is negligible vs
                # rs (>= hundreds). ln runs in-place in PSUM after the
                # rs_sb copy above has read the raw values.
                nc.scalar.activation(
                    out=rs_ps,
                    in_=rs_ps,
                    func=mybir.ActivationFunctionType.Ln,
                )
                rc_sb = rspool.tile([128, CHW], f32r, tag="rcsb", name="rcsb")
                nc.scalar.activation(
                    out=rc_sb,
                    in_=rs_ps,
                    func=mybir.ActivationFunctionType.Exp,
                    scale=-1.0,
                )
                for pair in range(2):
                    # pv += bv (x) rowsum   (rank-1 via K=64, rows 0 and 32)
                    nc.tensor.matmul(
                        pv_ps[pair],
                        bv_sb[64 * pair : 64 * (pair + 1), :],
                        rs_sb[64 * pair : 64 * (pair + 1), :],
                        start=False,
                        stop=True,
                    )
                    # rb = broadcast of 1/(rs+eps) to the pair's 128 rows
                    rb = rsps.tile([128, CHW], f32, tag="rspst", name="rb")
                    nc.tensor.matmul(
                        rb,
                        ones_sb[64 * pair : 64 * (pair + 1), :],
                        rc_sb[64 * pair : 64 * (pair + 1), :],
                        start=True,
                        stop=True,
                    )
                    rb_sb = outsb.tile([128, CHW], f32, tag="rbsb", name="rbsb")
                    nc.vector.tensor_copy(rb_sb, rb)
                    osb = outsb.tile([128, CHW], f32, tag="o", name="osb")
                    nc.vector.tensor_mul(osb, pv_ps[pair], rb_sb)
                    nc.sync.dma_start(o[128 * pair : 128 * (pair + 1), cs], osb)

            def new_pv_tiles():
                pv_ps = [
                    pvps.tile([128, CHW], f32, tag="pvpst", name=f"pv{i}")
                    for i in range(2)
                ]
                rs_ps = rsps.tile([128, CHW], f32, tag="rspst", name="rsps_t")
                return pv_ps, rs_ps

            proj_v()

            # all chunks fully interleaved; chunk c+1's q-projection is
            # emitted between chunk c's PV drain and its tail so the PE has
            # work while the rowsum->reciprocal DVE chain runs
            pending_tail = None
            for ch in range(NCH):
                cs = slice(ch * CHW, (ch + 1) * CHW)
                p_tiles = {}
                pv_ps, rs_ps = new_pv_tiles()
                LAG = 3
                for t in range(NKT + LAG):
                    if t < NKT:
                        for pair in range(2):
                            scores_group(pair, t, cs, p_tiles)
                    if t == 1 and pending_tail is not None:
                        pending_tail()
                        pending_tail = None
                    if t >= LAG:
                        pv_t(t - LAG, p_tiles, pv_ps, rs_ps)
                if ch + 1 < NCH:
                    proj_qk_chunk("q", q_later, ch + 1, qtT, bq_sb, cast_dma=True)

                def _tail(cs=cs, pv_ps=pv_ps, rs_ps=rs_ps):
                    chunk_tail(cs, pv_ps, rs_ps)

                pending_tail = _tail
            pending_tail()

    nc.compile()
    return nc


_NC = None


def _get_nc():
    global _NC
    if _NC is None:
        _NC = _build()
    return _NC


def _shard(inputs):
    import ml_dtypes

    q, k, v = inputs["q"], inputs["k"], inputs["v"]
    mask = inputs["mask"]
    Wq, bq, Wk, bk, Wv, bv = (
        inputs[n] for n in ("Wq", "bq", "Wk", "bk", "Wv", "bv")
    )
    qT = [np.ascontiguousarray(np.asarray(q[b], np.float32).T) for b in range(B)]
    kT = [np.ascontiguousarray(np.asarray(k[b], np.float32).T) for b in range(B)]
    vT = [np.ascontiguousarray(np.asarray(v[b], np.float32).T) for b in range(B)]
    mT = [
        np.ascontiguousarray(np.asarray(mask[b]).T).view(np.uint8)
        for b in range(B)
    ]
    ones2 = np.zeros((128, 128), np.float32)
    for p in range(2):
        ones2[64 * p, 0:64] = 1.0
        ones2[64 * p + 32, 64:128] = 1.0
    onesp = np.ones((128, 32), ml_dtypes.bfloat16)
    in_maps = []
    for c in range(N_CORES):
        b, jg = divmod(c, N_CORES // B)
        j0 = jg * JW
        bvs = np.asarray(bv, np.float32)[j0 : j0 + JW]
        bv2 = np.zeros((128, 128), np.float32)
        for p in range(2):
            bv2[64 * p, 0:64] = bvs[128 * p : 128 * p + 64]
            bv2[64 * p + 32, 64:128] = bvs[128 * p + 64 : 128 * p + 128]
        in_maps.append(
            {
                "qT": qT[b],
                "kT": kT[b],
                "vT": vT[b],
                "maskT": mT[b],
                "wqT": np.ascontiguousarray(
                    np.asarray(Wq, np.float32)[j0 : j0 + JW, :].T
                ),
                "wkT": np.ascontiguousarray(
                    np.asarray(Wk, np.float32)[j0 : j0 + JW, :].T
                ),
                "wvT": np.ascontiguousarray(
                    np.asarray(Wv, np.float32)[j0 : j0 + JW, :].T
                ),
                "bq": np.asarray(bq, np.float32)[j0 : j0 + JW].reshape(2, 128),
                "bk": np.asarray(bk, np.float32)[j0 : j0 + JW].reshape(2, 128),
                "bv2": bv2,
                "ones2": ones2,
                "onesp": onesp,
            }
        )
    return in_maps


LAST_RESULT = None


def kernel(**inputs) -> np.ndarray:
    global LAST_RESULT
    nc = _get_nc()
    in_maps = _shard(inputs)
    trace = bool(int(os.environ.get("KTRACE", "0")))
    res = run_bass_kernel_spmd(
        nc,
        in_maps,
        core_ids=list(range(N_CORES)),
        trace=trace,
        trace_cores=[0] if trace else None,
    )
    LAST_RESULT = res
    out = np.empty((B, NQ, D), np.float32)
    for c in range(N_CORES):
        b, jg = divmod(c, N_CORES // B)
        j0 = jg * JW
        oc = res.results[c]["o"]  # [256, NQ] pair-major
        out[b, :, j0 : j0 + JW] = (
            oc.reshape(2, 2, DH, NQ).transpose(3, 0, 1, 2).reshape(NQ, JW)
        )
    return out


if __name__ == "__main__":
    if os.environ.get("KBUILD_ONLY"):
        import tempfile

        from concourse.bass_utils import compile_bass_kernel

        nc = _build()
        with tempfile.TemporaryDirectory() as td:
            compile_bass_kernel(nc, td)
        print("BUILD+COMPILE OK")



# revision 6
# speedup vs baseline: 1.0924x; 1.0924x over previous
"""Trainium2 Bass kernel for nn_Attention_48498770706573.

Fused QKV-projection + masked softmax attention, sharded over 8 NeuronCores:
data-parallel over batch (B=2), tensor-parallel over heads (16 -> 4 per
core). Each core computes its (batch, 4-head) shard end to end; the host
only slices/transposes/bf16-casts inputs (no arithmetic beyond dtype
rounding) and concatenates the disjoint output shards.

The kernel is ACT(exp)-bound: 128 exps of [128,1024] ~= 130us of Scalar
engine time. The structure therefore maximizes ACT occupancy:
  - inputs arrive pre-cast bf16 (half the DMA bytes, no on-chip casts),
  - a minimal pre-phase (k-proj chunk0 + q-proj chunk0) so the first
    exp fires ~10us in,
  - the remaining projections (k-proj chunks 1-3, all of v-proj) are
    interleaved into chunk 0's attention iterations, using the psum
    slots that are free at that point (pvps before PV starts, the
    rotating rsps utility slot after),
  - scores S^T[nk, nq] as bf16 matmuls, two heads row-packed via
    base_partition (concurrent in the PE array), exp on ACT straight
    out of PSUM (1/32 scale folded in), bool mask cast u8->bf16 via
    SWDGE cast-DMA and applied with one broadcast DVE multiply,
  - PV with p^T bf16 moving, heads col-packed (concurrent), row-sums
    via col-packed ones matmuls, V-bias as a rank-1 matmul and
    normalization via a rank-1 broadcast of 1/(rowsum+eps),
  - per-chunk tails (rowsum -> reciprocal -> normalize) deferred into
    the next chunk's first iterations; outputs written bf16 on the
    SWDGE queue to keep the input DMA queue clean.
"""

import os

import numpy as np

import concourse.bacc as bacc
import concourse.hw_specs as _hw_specs
import concourse.mybir as mybir
import concourse.tile as tile
from concourse.bass_utils import run_bass_kernel_spmd

# The kernel uses both Exp and Ln; steer both into the combined
# "natural_log_exp_and_others" ACT table set so chunk tails don't thrash
# the table RAM (~2.7us per switch). Dict order (= act_func_set_id) kept.
_orig_get_act_tables = _hw_specs.get_activation_tables


def _patched_get_act_tables(module_arch):
    exp = mybir.ActivationFunctionType.Exp
    ln = mybir.ActivationFunctionType.Ln
    out = {}
    for name, funcs in _orig_get_act_tables(module_arch).items():
        f = set(funcs)
        if name != "natural_log_exp_and_others":
            f.discard(exp)
            f.discard(ln)
        out[name] = f
    return out


_hw_specs.get_activation_tables = _patched_get_act_tables
bacc.get_activation_tables = _patched_get_act_tables

B, NQ, NK, D, H = 2, 2048, 2048, 1024, 16
DH = D // H  # 64
N_CORES = 8
HPC = H // (N_CORES // B)  # heads per core = 4
JW = HPC * DH  # per-core projection width = 256
NKT = NK // 128  # 16 nk tiles
NCH = 4  # nq chunks
CHW = NQ // NCH  # 512
DT = 8  # contraction d-tiles

f32 = mybir.dt.float32
f32r = mybir.dt.float32r
bf16 = mybir.dt.bfloat16
u8 = mybir.dt.uint8


def _build():
    nc = bacc.Bacc(
        "TRN2", target_bir_lowering=False, debug=False, num_devices=N_CORES
    )

    qT = nc.dram_tensor("qT", [D, NQ], bf16, kind="ExternalInput")
    kT = nc.dram_tensor("kT", [D, NK], bf16, kind="ExternalInput")
    vT = nc.dram_tensor("vT", [D, NK], bf16, kind="ExternalInput")
    maskT = nc.dram_tensor("maskT", [NK, NQ], u8, kind="ExternalInput")
    wqT = nc.dram_tensor("wqT", [D, JW], bf16, kind="ExternalInput")
    wkT = nc.dram_tensor("wkT", [D, JW], bf16, kind="ExternalInput")
    wvT = nc.dram_tensor("wvT", [D, JW], bf16, kind="ExternalInput")
    bqd = nc.dram_tensor("bq", [2, 128], f32, kind="ExternalInput")
    bkd = nc.dram_tensor("bk", [2, 128], f32, kind="ExternalInput")
    # bv2[64p + 0, 0:64] = bv[128p + dh], bv2[64p + 32, 64:128] = bv[...]
    bvd = nc.dram_tensor("bv2", [128, 128], f32r, kind="ExternalInput")
    # ones2[64p, 0:64] = 1, ones2[64p + 32, 64:128] = 1
    onesd = nc.dram_tensor("ones2", [128, 128], f32r, kind="ExternalInput")
    onespd = nc.dram_tensor("onesp", [128, 32], bf16, kind="ExternalInput")
    o = nc.dram_tensor("o", [2 * 128, NQ], bf16, kind="ExternalOutput")

    with tile.TileContext(nc) as tc:
        with (
            tc.tile_pool(name="consts", bufs=1) as consts,
            tc.tile_pool(name="kst", bufs=16) as kst,
            tc.tile_pool(name="qst", bufs=16) as qst,
            tc.tile_pool(name="vst", bufs=32) as vst,
            tc.tile_pool(name="m8pool", bufs=16) as m8pool,
            tc.tile_pool(name="mbpool", bufs=7) as mbpool,
            tc.tile_pool(name="projout", bufs=1) as projout,
            tc.tile_pool(name="ppool", bufs=12) as ppool,
            tc.tile_pool(name="rspool", bufs=2) as rspool,
            tc.tile_pool(name="outsb", bufs=4) as outsb,
            tc.tile_pool(name="sps", bufs=2, space="PSUM") as sps,
            tc.tile_pool(name="pvps", bufs=2, space="PSUM") as pvps,
            tc.tile_pool(name="rsps", bufs=2, space="PSUM") as rsps,
        ):
            # ---- small constants (front of DMA queue) ----
            bq_sb = consts.tile([128, 2], f32, tag="bq")
            bk_sb = consts.tile([128, 2], f32, tag="bk")
            for m in range(2):
                nc.sync.dma_start(
                    bq_sb[:, m : m + 1],
                    bqd[m : m + 1, :].rearrange("a b -> b a"),
                )
                nc.sync.dma_start(
                    bk_sb[:, m : m + 1],
                    bkd[m : m + 1, :].rearrange("a b -> b a"),
                )
            bv_sb = consts.tile([128, 128], f32r, tag="bv")
            nc.sync.dma_start(bv_sb, bvd[:])
            ones_sb = consts.tile([128, 128], f32r, tag="ones")
            nc.sync.dma_start(ones_sb, onesd[:])
            onesp_sb = consts.tile([128, 32], bf16, tag="onesp")
            nc.sync.dma_start(onesp_sb, onespd[:])

            def dma_w(name, dram):
                t = consts.tile([128, DT, JW], bf16, tag=f"w{name}", name="w")
                for d in range(DT):
                    nc.sync.dma_start(t[:, d], dram[d * 128 : (d + 1) * 128, :])
                return t

            def dma_x(src, ch, pool, tag):
                ts = []
                for d in range(DT):
                    x = pool.tile([128, CHW], bf16, tag=tag, name=tag)
                    nc.sync.dma_start(
                        x,
                        src[d * 128 : (d + 1) * 128, ch * CHW : (ch + 1) * CHW],
                    )
                    ts.append(x)
                return ts

            def dma_m(t):
                mt8 = m8pool.tile([128, NQ], u8, tag="m8", name="m8")
                # SWDGE queue: keeps the bulk input queue free for k/q/v
                nc.gpsimd.dma_start(mt8, maskT[t * 128 : (t + 1) * 128, :])
                return mt8

            # ---- bulk input DMAs, emitted in consumption order ----
            # sync (HWDGE) queue: weights + k/q/v; masks ride the SWDGE
            # queue (m0-m2 up front, the rest prefetched from inside the
            # chunk-0 loop) so they don't delay the critical k/q path.
            w_k = dma_w("k", wkT)
            k_x = {0: dma_x(kT, 0, kst, "kx")}
            w_q = dma_w("q", wqT)
            q_x = {0: dma_x(qT, 0, qst, "qx")}
            m8 = {t: dma_m(t) for t in range(3)}
            k_x[1] = dma_x(kT, 1, kst, "kx")
            w_v = dma_w("v", wvT)
            v_x = {0: dma_x(vT, 0, vst, "vx")}
            v_x[1] = dma_x(vT, 1, vst, "vx")
            k_x[2] = dma_x(kT, 2, kst, "kx")
            v_x[2] = dma_x(vT, 2, vst, "vx")
            k_x[3] = dma_x(kT, 3, kst, "kx")
            v_x[3] = dma_x(vT, 3, vst, "vx")
            q_x[1] = dma_x(qT, 1, qst, "qx")
            q_x[2] = dma_x(qT, 2, qst, "qx")
            q_x[3] = dma_x(qT, 3, qst, "qx")

            # ---- projection outputs (split per chunk for clean deps) ----
            ktTs = [
                projout.tile([128, 2, CHW], bf16, tag=f"ktT{c}", name="ktT")
                for c in range(NCH)
            ]
            qtTs = [
                projout.tile([128, 2, CHW], bf16, tag=f"qtT{c}", name="qtT")
                for c in range(NCH)
            ]
            # vts[g][:, a, :] = vt for nk-tile 2g+a
            vts = [
                projout.tile([128, 2, JW], bf16, tag=f"vt{g}", name="vt")
                for g in range(NKT // 2)
            ]

            def proj_qk_sps(w, xs, dst, bias):
                """q/k projection chunk through one 2-bank sps tile:
                m0 -> cols 0:CHW, m1 -> cols CHW:2CHW."""
                ps = sps.tile([128, 2 * CHW], f32, tag="s", name="pps")
                for d in range(DT):
                    for m in range(2):
                        nc.tensor.matmul(
                            ps[:, m * CHW : (m + 1) * CHW],
                            w[:, d, m * 128 : (m + 1) * 128],
                            xs[d],
                            start=(d == 0),
                            stop=(d == DT - 1),
                        )
                for m in range(2):
                    nc.vector.tensor_scalar_add(
                        dst[:, m, :],
                        ps[:, m * CHW : (m + 1) * CHW],
                        bias[:, m : m + 1],
                    )

            def proj_qk_m(w, xs, dst, bias, m, pool, nm):
                """One m-half of a q/k projection chunk through a single
                [128, CHW] psum tile from `pool`."""
                ps = pool.tile([128, CHW], f32, tag=nm, name="pps")
                for d in range(DT):
                    nc.tensor.matmul(
                        ps,
                        w[:, d, m * 128 : (m + 1) * 128],
                        xs[d],
                        start=(d == 0),
                        stop=(d == DT - 1),
                    )
                nc.vector.tensor_scalar_add(
                    dst[:, m, :], ps, bias[:, m : m + 1]
                )

            def vproj_pair(g):
                """vt for nk-tiles 2g, 2g+1 through the rsps utility slot."""
                ps = rsps.tile([128, CHW], f32, tag="rspst", name="vps")
                ps2 = ps.rearrange("p (a j) -> p a j", a=2)
                for d in range(DT):
                    for a in range(2):
                        t = 2 * g + a
                        ch, nn_ = divmod(t, 4)
                        # start=True clears the WHOLE bank (all 512 cols) in
                        # the written partitions, so only the very first
                        # matmul of the packed pair may set it; the a=1
                        # group's first matmul overwrites where has_written
                        # is clear (bank-wide clear reset its bits too).
                        nc.tensor.matmul(
                            ps2[:, a],
                            v_x[ch][d][:, nn_ * 128 : (nn_ + 1) * 128],
                            w_v[:, d, :],
                            start=(d == 0 and a == 0),
                            stop=(d == DT - 1),
                        )
                nc.vector.tensor_copy(vts[g][:], ps2)

            # ---- attention ----
            def scores_group(pair, t, ch, p_tiles):
                sp = sps.tile([128, 2 * CHW], f32, tag="s", name="sp")
                kc, tt = ktTs[t // 4], t % 4
                for hh in range(2):
                    nc.tensor.matmul(
                        sp[:, hh * CHW : (hh + 1) * CHW],
                        kc[
                            64 * hh : 64 * (hh + 1),
                            pair,
                            tt * 128 : (tt + 1) * 128,
                        ],
                        qtTs[ch][64 * hh : 64 * (hh + 1), pair, :],
                        start=True,
                        stop=True,
                    )
                p = ppool.tile([128, 2 * CHW], bf16, tag="p", name="p")
                nc.scalar.activation(
                    out=p,
                    in_=sp,
                    func=mybir.ActivationFunctionType.Exp,
                    scale=1.0 / 32.0,
                )
                if pair == 0:
                    mb = mbpool.tile([128, CHW], bf16, tag="mb", name="mb")
                    # SWDGE cast DMA u8 -> bf16 (frees GpSimd compute)
                    nc.gpsimd.dma_start(
                        mb, m8[t][:, ch * CHW : (ch + 1) * CHW]
                    )
                    p_tiles[("mb", t)] = mb
                else:
                    mb = p_tiles[("mb", t)]
                p3 = p.rearrange("p (h c) -> p h c", h=2)
                nc.vector.tensor_mul(
                    p3,
                    p3,
                    mb.rearrange("p (a c) -> p a c", a=1).to_broadcast(
                        (128, 2, CHW)
                    ),
                )
                p_tiles[(pair, t)] = p

            def pv_t(t, p_tiles, pv_ps, rs_ps):
                st, sp_ = t == 0, t == NKT - 1
                g, a = divmod(t, 2)
                for pair in range(2):
                    p = p_tiles[(pair, t)]
                    for hh in range(2):
                        nc.tensor.matmul(
                            pv_ps[pair][64 * hh : 64 * (hh + 1), :],
                            vts[g][
                                :,
                                a,
                                128 * pair + 64 * hh : 128 * pair
                                + 64 * (hh + 1),
                            ],
                            p[:, hh * CHW : (hh + 1) * CHW],
                            start=st,
                            stop=sp_,
                            tile_position=(0, 64 * hh),
                        )
                for pair in range(2):
                    p = p_tiles[(pair, t)]
                    for hh in range(2):
                        hg = 2 * pair + hh
                        nc.tensor.matmul(
                            rs_ps[32 * hg : 32 * hg + 32, :],
                            onesp_sb[:, 0:32],
                            p[:, hh * CHW : (hh + 1) * CHW],
                            start=st,
                            stop=sp_,
                            tile_position=(0, 32 * hg),
                        )

            def chunk_tail(cs, pv_ps, rs_ps):
                # rowsum -> +eps -> reciprocal (all 128 rows valid: the M=32
                # rowsum matmuls wrote 32 identical rows per head)
                rs_sb = rspool.tile([128, CHW], f32r, tag="rssb", name="rssb")
                nc.vector.tensor_scalar_add(rs_sb, rs_ps, 1e-6)
                # 1/rs via ACT exp(-ln(rs)): ~1.2us and mostly hidden in the
                # ACT stream's chunk-boundary slack; eps negligible vs rs.
                nc.scalar.activation(
                    out=rs_ps,
                    in_=rs_ps,
                    func=mybir.ActivationFunctionType.Ln,
                )
                rc_sb = rspool.tile([128, CHW], f32r, tag="rcsb", name="rcsb")
                nc.scalar.activation(
                    out=rc_sb,
                    in_=rs_ps,
                    func=mybir.ActivationFunctionType.Exp,
                    scale=-1.0,
                )
                for pair in range(2):
                    # pv += bv (x) rowsum   (rank-1 via K=64, rows 0 and 32)
                    nc.tensor.matmul(
                        pv_ps[pair],
                        bv_sb[64 * pair : 64 * (pair + 1), :],
                        rs_sb[64 * pair : 64 * (pair + 1), :],
                        start=False,
                        stop=True,
                    )
                    # rb = broadcast of 1/(rs+eps) to the pair's 128 rows
                    rb = rsps.tile([128, CHW], f32, tag="rspst", name="rb")
                    nc.tensor.matmul(
                        rb,
                        ones_sb[64 * pair : 64 * (pair + 1), :],
                        rc_sb[64 * pair : 64 * (pair + 1), :],
                        start=True,
                        stop=True,
                    )
                    rb_sb = outsb.tile([128, CHW], f32, tag="rbsb", name="rbsb")
                    nc.vector.tensor_copy(rb_sb, rb)
                    osb = outsb.tile([128, CHW], bf16, tag="o", name="osb")
                    nc.vector.tensor_mul(osb, pv_ps[pair], rb_sb)
                    nc.gpsimd.dma_start(
                        o[128 * pair : 128 * (pair + 1), cs], osb
                    )

            def new_pv_tiles():
                pv_ps = [
                    pvps.tile([128, CHW], f32, tag="pvpst", name=f"pv{i}")
                    for i in range(2)
                ]
                rs_ps = rsps.tile([128, CHW], f32, tag="rspst", name="rsps_t")
                return pv_ps, rs_ps

            # ---- pre-phase: k-proj chunk0 + q-proj chunk0 ----
            proj_qk_sps(w_k, k_x[0], ktTs[0], bk_sb)
            proj_qk_sps(w_q, q_x[0], qtTs[0], bq_sb)

            # chunk 0 utility-phase schedule (kc1 through the pvps slots
            # before PV claims them; everything else through the rsps
            # rotating slot). Keyed by iteration; placed so each phase's
            # inputs (DMA) land just before and its output is ready just
            # before its first consumer.
            util0 = {
                2: [lambda: proj_qk_m(w_k, k_x[1], ktTs[1], bk_sb, 0, pvps, "pvpst")],
                3: [
                    lambda: proj_qk_m(w_k, k_x[1], ktTs[1], bk_sb, 1, pvps, "pvpst"),
                    lambda: vproj_pair(0),
                ],
                4: [lambda: vproj_pair(1)],
                5: [lambda: vproj_pair(2)],
                6: [lambda: proj_qk_m(w_k, k_x[2], ktTs[2], bk_sb, 0, rsps, "rspst")],
                7: [lambda: proj_qk_m(w_k, k_x[2], ktTs[2], bk_sb, 1, rsps, "rspst")],
                8: [lambda: vproj_pair(3)],
                9: [lambda: vproj_pair(4)],
                10: [lambda: proj_qk_m(w_k, k_x[3], ktTs[3], bk_sb, 0, rsps, "rspst")],
                11: [lambda: proj_qk_m(w_k, k_x[3], ktTs[3], bk_sb, 1, rsps, "rspst")],
                12: [lambda: vproj_pair(5)],
                13: [lambda: vproj_pair(6)],
                14: [lambda: vproj_pair(7)],
            }

            # all chunks fully interleaved; chunk c+1's q-projection is
            # emitted between chunk c's PV drain and its tail so the PE has
            # work while the rowsum->reciprocal chain runs
            pending_tail = None
            LAG = 4
            for ch in range(NCH):
                cs = slice(ch * CHW, (ch + 1) * CHW)
                p_tiles = {}
                pv_ps, rs_ps = None, None
                for t in range(NKT + LAG):
                    if ch == 0 and t in util0:
                        for fn in util0[t]:
                            fn()
                    if ch == 0 and t == 3:
                        # pvps slots freed by the kc1 drains; claim for PV
                        pv_ps, rs_ps = new_pv_tiles()
                    elif ch > 0 and t == 0:
                        pv_ps, rs_ps = new_pv_tiles()
                    if t < NKT:
                        for pair in range(2):
                            scores_group(pair, t, ch, p_tiles)
                        if ch == 0 and t + 3 < NKT:
                            # prefetch mask tiles on the SWDGE queue,
                            # 3 iterations ahead of first use
                            m8[t + 3] = dma_m(t + 3)
                    if t == 1 and pending_tail is not None:
                        pending_tail()
                        pending_tail = None
                    if t >= LAG:
                        pv_t(t - LAG, p_tiles, pv_ps, rs_ps)
                if ch + 1 < NCH:
                    for m in range(2):
                        proj_qk_m(
                            w_q, q_x[ch + 1], qtTs[ch + 1], bq_sb, m,
                            rsps, "rspst",
                        )

                def _tail(cs=cs, pv_ps=pv_ps, rs_ps=rs_ps):
                    chunk_tail(cs, pv_ps, rs_ps)

                pending_tail = _tail
            pending_tail()

    nc.compile()
    return nc


_NC = None


def _get_nc():
    global _NC
    if _NC is None:
        _NC = _build()
    return _NC


def _shard(inputs):
    import ml_dtypes

    bfl = ml_dtypes.bfloat16
    q, k, v = inputs["q"], inputs["k"], inputs["v"]
    mask = inputs["mask"]
    Wq, bq, Wk, bk, Wv, bv = (
        inputs[n] for n in ("Wq", "bq", "Wk", "bk", "Wv", "bv")
    )
    qT = [np.ascontiguousarray(np.asarray(q[b]).T.astype(bfl)) for b in range(B)]
    kT = [np.ascontiguousarray(np.asarray(k[b]).T.astype(bfl)) for b in range(B)]
    vT = [np.ascontiguousarray(np.asarray(v[b]).T.astype(bfl)) for b in range(B)]
    mT = [
        np.ascontiguousarray(np.asarray(mask[b]).T).view(np.uint8)
        for b in range(B)
    ]
    ones2 = np.zeros((128, 128), np.float32)
    for p in range(2):
        ones2[64 * p, 0:64] = 1.0
        ones2[64 * p + 32, 64:128] = 1.0
    onesp = np.ones((128, 32), bfl)
    in_maps = []
    for c in range(N_CORES):
        b, jg = divmod(c, N_CORES // B)
        j0 = jg * JW
        bvs = np.asarray(bv, np.float32)[j0 : j0 + JW]
        bv2 = np.zeros((128, 128), np.float32)
        for p in range(2):
            bv2[64 * p, 0:64] = bvs[128 * p : 128 * p + 64]
            bv2[64 * p + 32, 64:128] = bvs[128 * p + 64 : 128 * p + 128]
        in_maps.append(
            {
                "qT": qT[b],
                "kT": kT[b],
                "vT": vT[b],
                "maskT": mT[b],
                "wqT": np.ascontiguousarray(
                    np.asarray(Wq)[j0 : j0 + JW, :].T.astype(bfl)
                ),
                "wkT": np.ascontiguousarray(
                    np.asarray(Wk)[j0 : j0 + JW, :].T.astype(bfl)
                ),
                "wvT": np.ascontiguousarray(
                    np.asarray(Wv)[j0 : j0 + JW, :].T.astype(bfl)
                ),
                "bq": np.asarray(bq, np.float32)[j0 : j0 + JW].reshape(2, 128),
                "bk": np.asarray(bk, np.float32)[j0 : j0 + JW].reshape(2, 128),
                "bv2": bv2,
                "ones2": ones2,
                "onesp": onesp,
            }
        )
    return in_maps


LAST_RESULT = None


def kernel(**inputs) -> np.ndarray:
    global LAST_RESULT
    nc = _get_nc()
    in_maps = _shard(inputs)
    trace = bool(int(os.environ.get("KTRACE", "0")))
    res = run_bass_kernel_spmd(
        nc,
        in_maps,
        core_ids=list(range(N_CORES)),
        trace=trace,
        trace_cores=[0] if trace else None,
    )
    LAST_RESULT = res
    out = np.empty((B, NQ, D), np.float32)
    for c in range(N_CORES):
        b, jg = divmod(c, N_CORES // B)
        j0 = jg * JW
        oc = res.results[c]["o"].astype(np.float32)  # [256, NQ] pair-major
        out[b, :, j0 : j0 + JW] = (
            oc.reshape(2, 2, DH, NQ).transpose(3, 0, 1, 2).reshape(NQ, JW)
        )
    return out


if __name__ == "__main__":
    if os.environ.get("KBUILD_ONLY"):
        import tempfile

        from concourse.bass_utils import compile_bass_kernel

        nc = _build()
        with tempfile.TemporaryDirectory() as td:
            compile_bass_kernel(nc, td)
        print("BUILD+COMPILE OK")


# revision 7
# speedup vs baseline: 1.1680x; 1.0692x over previous
"""Trainium2 Bass kernel for nn_Attention_48498770706573.

Fused QKV-projection + masked softmax attention, sharded over 8 NeuronCores:
data-parallel over batch (B=2), tensor-parallel over heads (16 -> 4 per
core). Each core computes its (batch, 4-head) shard end to end; the host
only slices/transposes/bf16-casts inputs (no arithmetic beyond dtype
rounding) and concatenates the disjoint output shards.

The kernel is ACT(exp)-bound: 128 exps of [128,1024] ~= 130us of Scalar
engine time. The structure maximizes ACT occupancy:
  - inputs arrive pre-cast bf16 in partition-major chunk layouts so each
    k/q/v chunk is ONE dma_start (128 descriptors x 8KB) -- the DMA
    queue issues in ~1us instead of ~5us per chunk,
  - a minimal pre-phase (k-proj chunk0 + q-proj chunk0) so the first
    exp fires ~10us in,
  - the remaining projections (k-proj chunks 1-3, all of v-proj) are
    interleaved into chunk 0's attention iterations through the psum
    slots that are free at that point (pvps before PV starts, the
    rotating rsps utility slot after),
  - scores S^T[nk, nq] as bf16 matmuls, two heads row-packed via
    base_partition (concurrent in the PE array), exp on ACT straight
    out of PSUM (1/32 scale folded in), bool mask cast u8->bf16 via
    SWDGE cast-DMA and applied with one broadcast DVE multiply,
  - PV with p^T bf16 moving, heads col-packed (concurrent), row-sums
    via col-packed ones matmuls, V-bias as a rank-1 matmul and
    normalization via a rank-1 broadcast of 1/(rowsum+eps); the
    reciprocal runs on DVE (reciprocal_approx_fast) so chunk tails
    never block the ACT queue,
  - per-chunk tails deferred into the next chunk's first iterations;
    next-chunk q-projection emitted near the end of the current chunk;
    outputs written bf16 on the SWDGE queue to keep the input queue
    clean.
"""

import os

import numpy as np

import concourse.bacc as bacc
import concourse.mybir as mybir
import concourse.tile as tile
from concourse.bass_utils import run_bass_kernel_spmd

B, NQ, NK, D, H = 2, 2048, 2048, 1024, 16
DH = D // H  # 64
N_CORES = 8
HPC = H // (N_CORES // B)  # heads per core = 4
JW = HPC * DH  # per-core projection width = 256
NKT = NK // 128  # 16 nk tiles
NCH = 4  # nq chunks
CHW = NQ // NCH  # 512
DT = 8  # contraction d-tiles

f32 = mybir.dt.float32
bf16 = mybir.dt.bfloat16
u8 = mybir.dt.uint8


def _build():
    nc = bacc.Bacc(
        "TRN2", target_bir_lowering=False, debug=False, num_devices=N_CORES
    )

    # x tensors in partition-major chunk layout: X[p, ch, d, n] =
    # x[ch*CHW + n, d*128 + p] -- one contiguous 8KB run per partition
    # per chunk, so a chunk is a single 128-descriptor dma_start.
    qTd = nc.dram_tensor("qT", [128, NCH, DT, CHW], bf16, kind="ExternalInput")
    kTd = nc.dram_tensor("kT", [128, NCH, DT, CHW], bf16, kind="ExternalInput")
    vTd = nc.dram_tensor("vT", [128, NCH, DT, CHW], bf16, kind="ExternalInput")
    # mask: M[p, t, n] = mask[n, t*128 + p]
    maskd = nc.dram_tensor("maskT", [128, NKT, NQ], u8, kind="ExternalInput")
    # weights: W[p, d, j] = w[d*128 + p, j]
    wqd = nc.dram_tensor("wqT", [128, DT, JW], bf16, kind="ExternalInput")
    wkd = nc.dram_tensor("wkT", [128, DT, JW], bf16, kind="ExternalInput")
    wvd = nc.dram_tensor("wvT", [128, DT, JW], bf16, kind="ExternalInput")
    bqd = nc.dram_tensor("bq", [128, 2], f32, kind="ExternalInput")
    bkd = nc.dram_tensor("bk", [128, 2], f32, kind="ExternalInput")
    # bv2[64p + 0, 0:64] = bv[128p + dh], bv2[64p + 32, 64:128] = bv[...]
    bvd = nc.dram_tensor("bv2", [128, 128], f32, kind="ExternalInput")
    # ones2[64p, 0:64] = 1, ones2[64p + 32, 64:128] = 1
    onesd = nc.dram_tensor("ones2", [128, 128], f32, kind="ExternalInput")
    onespd = nc.dram_tensor("onesp", [128, 32], bf16, kind="ExternalInput")
    o = nc.dram_tensor("o", [2 * 128, NQ], bf16, kind="ExternalOutput")

    with tile.TileContext(nc) as tc:
        with (
            tc.tile_pool(name="consts", bufs=1) as consts,
            tc.tile_pool(name="kst", bufs=2) as kst,
            tc.tile_pool(name="qst", bufs=2) as qst,
            tc.tile_pool(name="vst", bufs=4) as vst,
            tc.tile_pool(name="m8pool", bufs=4) as m8pool,
            tc.tile_pool(name="mbpool", bufs=7) as mbpool,
            tc.tile_pool(name="projout", bufs=1) as projout,
            tc.tile_pool(name="ppool", bufs=12) as ppool,
            tc.tile_pool(name="rspool", bufs=2) as rspool,
            tc.tile_pool(name="outsb", bufs=4) as outsb,
            tc.tile_pool(name="sps", bufs=2, space="PSUM") as sps,
            tc.tile_pool(name="pvps", bufs=2, space="PSUM") as pvps,
            tc.tile_pool(name="rsps", bufs=2, space="PSUM") as rsps,
        ):
            def dma_w(name, dram):
                t = consts.tile([128, DT, JW], bf16, tag=f"w{name}", name="w")
                nc.sync.dma_start(t, dram[:])
                return t

            def dma_x(src, ch, pool, tag):
                x = pool.tile([128, DT, CHW], bf16, tag=tag, name=tag)
                nc.sync.dma_start(x, src[:, ch])
                return x

            def dma_m(g):
                mt8 = m8pool.tile([128, 4, NQ], u8, tag="m8", name="m8")
                # SWDGE queue: keeps the bulk input queue free for k/q/v
                nc.gpsimd.dma_start(mt8, maskd[:, 4 * g : 4 * g + 4, :])
                return mt8

            # ---- input DMAs, emitted in consumption order ----
            bq_sb = consts.tile([128, 2], f32, tag="bq")
            nc.sync.dma_start(bq_sb, bqd[:])
            bk_sb = consts.tile([128, 2], f32, tag="bk")
            nc.sync.dma_start(bk_sb, bkd[:])
            onesp_sb = consts.tile([128, 32], bf16, tag="onesp")
            nc.sync.dma_start(onesp_sb, onespd[:])
            w_k = dma_w("k", wkd)
            k_x = {0: dma_x(kTd, 0, kst, "kx")}
            w_q = dma_w("q", wqd)
            q_x = {0: dma_x(qTd, 0, qst, "qx")}
            m8 = [dma_m(0), dma_m(1)]
            k_x[1] = dma_x(kTd, 1, kst, "kx")
            w_v = dma_w("v", wvd)
            v_x = {0: dma_x(vTd, 0, vst, "vx")}
            v_x[1] = dma_x(vTd, 1, vst, "vx")
            m8 += [dma_m(2), dma_m(3)]
            k_x[2] = dma_x(kTd, 2, kst, "kx")
            v_x[2] = dma_x(vTd, 2, vst, "vx")
            k_x[3] = dma_x(kTd, 3, kst, "kx")
            v_x[3] = dma_x(vTd, 3, vst, "vx")
            bv_sb = consts.tile([128, 128], f32, tag="bv")
            nc.sync.dma_start(bv_sb, bvd[:])
            ones_sb = consts.tile([128, 128], f32, tag="ones")
            nc.sync.dma_start(ones_sb, onesd[:])
            q_x[1] = dma_x(qTd, 1, qst, "qx")
            q_x[2] = dma_x(qTd, 2, qst, "qx")
            q_x[3] = dma_x(qTd, 3, qst, "qx")

            # ---- projection outputs (split per chunk for clean deps) ----
            ktTs = [
                projout.tile([128, 2, CHW], bf16, tag=f"ktT{c}", name="ktT")
                for c in range(NCH)
            ]
            qtTs = [
                projout.tile([128, 2, CHW], bf16, tag=f"qtT{c}", name="qtT")
                for c in range(NCH)
            ]
            # vts[g][:, a, :] = vt for nk-tile 2g+a
            vts = [
                projout.tile([128, 2, JW], bf16, tag=f"vt{g}", name="vt")
                for g in range(NKT // 2)
            ]

            def proj_qk_sps(w, xs, dst, bias):
                """q/k projection chunk through one 2-bank sps tile:
                m0 -> cols 0:CHW, m1 -> cols CHW:2CHW."""
                ps = sps.tile([128, 2 * CHW], f32, tag="s", name="pps")
                for d in range(DT):
                    for m in range(2):
                        nc.tensor.matmul(
                            ps[:, m * CHW : (m + 1) * CHW],
                            w[:, d, m * 128 : (m + 1) * 128],
                            xs[:, d],
                            start=(d == 0),
                            stop=(d == DT - 1),
                        )
                for m in range(2):
                    nc.vector.tensor_scalar_add(
                        dst[:, m, :],
                        ps[:, m * CHW : (m + 1) * CHW],
                        bias[:, m : m + 1],
                    )

            def proj_qk_m(w, xs, dst, bias, m, pool, nm):
                """One m-half of a q/k projection chunk through a single
                [128, CHW] psum tile from `pool`."""
                ps = pool.tile([128, CHW], f32, tag=nm, name="pps")
                for d in range(DT):
                    nc.tensor.matmul(
                        ps,
                        w[:, d, m * 128 : (m + 1) * 128],
                        xs[:, d],
                        start=(d == 0),
                        stop=(d == DT - 1),
                    )
                nc.vector.tensor_scalar_add(
                    dst[:, m, :], ps, bias[:, m : m + 1]
                )

            def vproj_pair(g):
                """vt for nk-tiles 2g, 2g+1 through the rsps utility slot."""
                ps = rsps.tile([128, CHW], f32, tag="rspst", name="vps")
                ps2 = ps.rearrange("p (a j) -> p a j", a=2)
                for d in range(DT):
                    for a in range(2):
                        t = 2 * g + a
                        ch, nn_ = divmod(t, 4)
                        # start=True clears the WHOLE bank (all 512 cols) in
                        # the written partitions, so only the very first
                        # matmul of the packed pair may set it; the a=1
                        # group's first matmul overwrites where has_written
                        # is clear (bank-wide clear reset its bits too).
                        nc.tensor.matmul(
                            ps2[:, a],
                            v_x[ch][:, d, nn_ * 128 : (nn_ + 1) * 128],
                            w_v[:, d, :],
                            start=(d == 0 and a == 0),
                            stop=(d == DT - 1),
                        )
                nc.vector.tensor_copy(vts[g][:], ps2)

            # ---- attention ----
            def scores_group(pair, t, ch, p_tiles):
                sp = sps.tile([128, 2 * CHW], f32, tag="s", name="sp")
                kc, tt = ktTs[t // 4], t % 4
                for hh in range(2):
                    nc.tensor.matmul(
                        sp[:, hh * CHW : (hh + 1) * CHW],
                        kc[
                            64 * hh : 64 * (hh + 1),
                            pair,
                            tt * 128 : (tt + 1) * 128,
                        ],
                        qtTs[ch][64 * hh : 64 * (hh + 1), pair, :],
                        start=True,
                        stop=True,
                    )
                p = ppool.tile([128, 2 * CHW], bf16, tag="p", name="p")
                nc.scalar.activation(
                    out=p,
                    in_=sp,
                    func=mybir.ActivationFunctionType.Exp,
                    scale=1.0 / 32.0,
                )
                if pair == 0:
                    mb = mbpool.tile([128, CHW], bf16, tag="mb", name="mb")
                    # SWDGE cast DMA u8 -> bf16 (frees GpSimd compute)
                    nc.gpsimd.dma_start(
                        mb, m8[t // 4][:, t % 4, ch * CHW : (ch + 1) * CHW]
                    )
                    p_tiles[("mb", t)] = mb
                else:
                    mb = p_tiles[("mb", t)]
                p3 = p.rearrange("p (h c) -> p h c", h=2)
                nc.vector.tensor_mul(
                    p3,
                    p3,
                    mb.rearrange("p (a c) -> p a c", a=1).to_broadcast(
                        (128, 2, CHW)
                    ),
                )
                p_tiles[(pair, t)] = p

            def pv_t(t, p_tiles, pv_ps, rs_ps):
                st, sp_ = t == 0, t == NKT - 1
                g, a = divmod(t, 2)
                for pair in range(2):
                    p = p_tiles[(pair, t)]
                    for hh in range(2):
                        nc.tensor.matmul(
                            pv_ps[pair][64 * hh : 64 * (hh + 1), :],
                            vts[g][
                                :,
                                a,
                                128 * pair + 64 * hh : 128 * pair
                                + 64 * (hh + 1),
                            ],
                            p[:, hh * CHW : (hh + 1) * CHW],
                            start=st,
                            stop=sp_,
                            tile_position=(0, 64 * hh),
                        )
                for pair in range(2):
                    p = p_tiles[(pair, t)]
                    for hh in range(2):
                        hg = 2 * pair + hh
                        nc.tensor.matmul(
                            rs_ps[32 * hg : 32 * hg + 32, :],
                            onesp_sb[:, 0:32],
                            p[:, hh * CHW : (hh + 1) * CHW],
                            start=st,
                            stop=sp_,
                            tile_position=(0, 32 * hg),
                        )

            def chunk_tail(cs, pv_ps, rs_ps):
                # rowsum -> +eps -> reciprocal (all 128 rows valid: the M=32
                # rowsum matmuls wrote 32 identical rows per head). The
                # reciprocal runs on DVE so it never blocks the ACT stream.
                rs_sb = rspool.tile([128, CHW], f32, tag="rssb", name="rssb")
                nc.vector.tensor_scalar_add(rs_sb, rs_ps, 1e-6)
                rc_sb = rspool.tile([128, CHW], f32, tag="rcsb", name="rcsb")
                nc.vector.reciprocal_approx_fast(out=rc_sb, in_=rs_sb)
                for pair in range(2):
                    # pv += bv (x) rowsum   (rank-1 via K=64, rows 0 and 32)
                    nc.tensor.matmul(
                        pv_ps[pair],
                        bv_sb[64 * pair : 64 * (pair + 1), :],
                        rs_sb[64 * pair : 64 * (pair + 1), :],
                        start=False,
                        stop=True,
                    )
                    # rb = broadcast of 1/(rs+eps) to the pair's 128 rows
                    rb = rsps.tile([128, CHW], f32, tag="rspst", name="rb")
                    nc.tensor.matmul(
                        rb,
                        ones_sb[64 * pair : 64 * (pair + 1), :],
                        rc_sb[64 * pair : 64 * (pair + 1), :],
                        start=True,
                        stop=True,
                    )
                    rb_sb = outsb.tile([128, CHW], f32, tag="rbsb", name="rbsb")
                    nc.vector.tensor_copy(rb_sb, rb)
                    osb = outsb.tile([128, CHW], bf16, tag="o", name="osb")
                    nc.vector.tensor_mul(osb, pv_ps[pair], rb_sb)
                    nc.gpsimd.dma_start(
                        o[128 * pair : 128 * (pair + 1), cs], osb
                    )

            def new_pv_tiles():
                pv_ps = [
                    pvps.tile([128, CHW], f32, tag="pvpst", name=f"pv{i}")
                    for i in range(2)
                ]
                rs_ps = rsps.tile([128, CHW], f32, tag="rspst", name="rsps_t")
                return pv_ps, rs_ps

            # ---- pre-phase: k-proj chunk0 + q-proj chunk0 ----
            proj_qk_sps(w_k, k_x[0], ktTs[0], bk_sb)
            proj_qk_sps(w_q, q_x[0], qtTs[0], bq_sb)

            # chunk 0 utility-phase schedule (kc1 through the pvps slots
            # before PV claims them; everything else through the rsps
            # rotating slot). Keyed by iteration; placed so each phase's
            # inputs (DMA) land just before and its output is ready just
            # before its first consumer.
            util0 = {
                2: [lambda: proj_qk_m(w_k, k_x[1], ktTs[1], bk_sb, 0, pvps, "pvpst")],
                3: [
                    lambda: proj_qk_m(w_k, k_x[1], ktTs[1], bk_sb, 1, pvps, "pvpst"),
                    lambda: vproj_pair(0),
                ],
                4: [lambda: vproj_pair(1)],
                5: [lambda: vproj_pair(2)],
                6: [lambda: proj_qk_m(w_k, k_x[2], ktTs[2], bk_sb, 0, rsps, "rspst")],
                7: [lambda: proj_qk_m(w_k, k_x[2], ktTs[2], bk_sb, 1, rsps, "rspst")],
                8: [lambda: vproj_pair(3)],
                9: [lambda: vproj_pair(4)],
                10: [lambda: proj_qk_m(w_k, k_x[3], ktTs[3], bk_sb, 0, rsps, "rspst")],
                11: [lambda: proj_qk_m(w_k, k_x[3], ktTs[3], bk_sb, 1, rsps, "rspst")],
                12: [lambda: vproj_pair(5)],
                13: [lambda: vproj_pair(6)],
                14: [lambda: vproj_pair(7)],
            }

            # all chunks fully interleaved; chunk c+1's q-projection is
            # emitted near the end of chunk c (utility slot) so the next
            # chunk's scores can start the moment chunk c's last exp ends
            pending_tail = None
            LAG = 4
            for ch in range(NCH):
                cs = slice(ch * CHW, (ch + 1) * CHW)
                p_tiles = {}
                pv_ps, rs_ps = None, None
                qp_iters = (15, 16) if ch == 0 else (13, 14)
                for t in range(NKT + LAG):
                    if ch == 0 and t in util0:
                        for fn in util0[t]:
                            fn()
                    if ch == 0 and t == 3:
                        # pvps slots freed by the kc1 drains; claim for PV
                        pv_ps, rs_ps = new_pv_tiles()
                    elif ch > 0 and t == 0:
                        pv_ps, rs_ps = new_pv_tiles()
                    if t < NKT:
                        for pair in range(2):
                            scores_group(pair, t, ch, p_tiles)
                    if t == 1 and pending_tail is not None:
                        pending_tail()
                        pending_tail = None
                    if t >= LAG:
                        pv_t(t - LAG, p_tiles, pv_ps, rs_ps)
                    if ch + 1 < NCH and t in qp_iters:
                        proj_qk_m(
                            w_q, q_x[ch + 1], qtTs[ch + 1], bq_sb,
                            qp_iters.index(t), rsps, "rspst",
                        )

                def _tail(cs=cs, pv_ps=pv_ps, rs_ps=rs_ps):
                    chunk_tail(cs, pv_ps, rs_ps)

                pending_tail = _tail
            pending_tail()

    nc.compile()
    return nc


_NC = None


def _get_nc():
    global _NC
    if _NC is None:
        _NC = _build()
    return _NC


def _shard(inputs):
    import ml_dtypes

    bfl = ml_dtypes.bfloat16
    q, k, v = inputs["q"], inputs["k"], inputs["v"]
    mask = inputs["mask"]
    Wq, bq, Wk, bk, Wv, bv = (
        inputs[n] for n in ("Wq", "bq", "Wk", "bk", "Wv", "bv")
    )

    def xfmt(x):
        # [N, D] -> [128, NCH, DT, CHW]: X[p, ch, d, n] = x[ch*CHW+n, 128d+p]
        return np.ascontiguousarray(
            np.asarray(x)
            .reshape(NCH, CHW, DT, 128)
            .transpose(3, 0, 2, 1)
            .astype(bfl)
        )

    def wfmt(w, j0):
        # [D, D] -> [128, DT, JW]: W[p, d, j] = w[j0+j, 128d+p]
        return np.ascontiguousarray(
            np.asarray(w)[j0 : j0 + JW, :].T.reshape(DT, 128, JW)
            .transpose(1, 0, 2)
            .astype(bfl)
        )

    qX = [xfmt(q[b]) for b in range(B)]
    kX = [xfmt(k[b]) for b in range(B)]
    vX = [xfmt(v[b]) for b in range(B)]
    # mask [NQ, NK] bool -> [128, NKT, NQ] u8: M[p, t, n] = mask[n, 128t+p]
    mX = [
        np.ascontiguousarray(
            np.asarray(mask[b]).T.reshape(NKT, 128, NQ).transpose(1, 0, 2)
        ).view(np.uint8)
        for b in range(B)
    ]
    ones2 = np.zeros((128, 128), np.float32)
    for p in range(2):
        ones2[64 * p, 0:64] = 1.0
        ones2[64 * p + 32, 64:128] = 1.0
    onesp = np.ones((128, 32), bfl)
    in_maps = []
    for c in range(N_CORES):
        b, jg = divmod(c, N_CORES // B)
        j0 = jg * JW
        bvs = np.asarray(bv, np.float32)[j0 : j0 + JW]
        bv2 = np.zeros((128, 128), np.float32)
        for p in range(2):
            bv2[64 * p, 0:64] = bvs[128 * p : 128 * p + 64]
            bv2[64 * p + 32, 64:128] = bvs[128 * p + 64 : 128 * p + 128]
        in_maps.append(
            {
                "qT": qX[b],
                "kT": kX[b],
                "vT": vX[b],
                "maskT": mX[b],
                "wqT": wfmt(Wq, j0),
                "wkT": wfmt(Wk, j0),
                "wvT": wfmt(Wv, j0),
                "bq": np.ascontiguousarray(
                    np.asarray(bq, np.float32)[j0 : j0 + JW].reshape(2, 128).T
                ),
                "bk": np.ascontiguousarray(
                    np.asarray(bk, np.float32)[j0 : j0 + JW].reshape(2, 128).T
                ),
                "bv2": bv2,
                "ones2": ones2,
                "onesp": onesp,
            }
        )
    return in_maps


LAST_RESULT = None


def kernel(**inputs) -> np.ndarray:
    global LAST_RESULT
    nc = _get_nc()
    in_maps = _shard(inputs)
    trace = bool(int(os.environ.get("KTRACE", "0")))
    res = run_bass_kernel_spmd(
        nc,
        in_maps,
        core_ids=list(range(N_CORES)),
        trace=trace,
        trace_cores=[0] if trace else None,
    )
    LAST_RESULT = res
    out = np.empty((B, NQ, D), np.float32)
    for c in range(N_CORES):
        b, jg = divmod(c, N_CORES // B)
        j0 = jg * JW
        oc = res.results[c]["o"].astype(np.float32)  # [256, NQ] pair-major
        out[b, :, j0 : j0 + JW] = (
            oc.reshape(2, 2, DH, NQ).transpose(3, 0, 1, 2).reshape(NQ, JW)
        )
    return out


if __name__ == "__main__":
    if os.environ.get("KBUILD_ONLY"):
        import tempfile

        from concourse.bass_utils import compile_bass_kernel

        nc = _build()
        with tempfile.TemporaryDirectory() as td:
            compile_bass_kernel(nc, td)
        print("BUILD+COMPILE OK")


# revision 12
# speedup vs baseline: 1.2029x; 1.0299x over previous
"""Trainium2 Bass kernel for nn_Attention_48498770706573.

Fused QKV-projection + masked softmax attention, sharded over 8 NeuronCores:
data-parallel over batch (B=2), tensor-parallel over heads (16 -> 4 per
core). Each core computes its (batch, 4-head) shard end to end; the host
only slices/transposes/bf16-casts inputs (no arithmetic beyond dtype
rounding) and concatenates the disjoint output shards.

The kernel is ACT(exp)-bound: 128 exps of [128,1024] ~= 130us of Scalar
engine time. The structure maximizes ACT occupancy:
  - inputs arrive pre-cast bf16 in partition-major chunk layouts so each
    k/q/v chunk is ONE dma_start (128 descriptors x 8KB) -- the DMA
    queue issues in ~1us instead of ~5us per chunk,
  - a minimal pre-phase (k-proj chunk0 + q-proj chunk0) so the first
    exp fires ~10us in,
  - the remaining projections (k-proj chunks 1-3, all of v-proj) are
    interleaved into chunk 0's attention iterations through the psum
    slots that are free at that point (pvps before PV starts, the
    rotating rsps utility slot after),
  - scores S^T[nk, nq] as bf16 matmuls, two heads row-packed via
    base_partition (concurrent in the PE array), exp on ACT straight
    out of PSUM (1/32 scale folded in), bool mask cast u8->bf16 via
    SWDGE cast-DMA and applied with one broadcast DVE multiply,
  - PV with p^T bf16 moving, heads col-packed (concurrent), row-sums
    via col-packed ones matmuls, V-bias as a rank-1 matmul and
    normalization via a rank-1 broadcast of 1/(rowsum+eps); the
    reciprocal runs on DVE (reciprocal_approx_fast) so chunk tails
    never block the ACT queue,
  - per-chunk tails deferred into the next chunk's first iterations;
    next-chunk q-projection emitted near the end of the current chunk;
    outputs written bf16 on the SWDGE queue to keep the input queue
    clean.
"""

import os

import numpy as np

import concourse.bacc as bacc
import concourse.mybir as mybir
import concourse.tile as tile
from concourse.bass_utils import run_bass_kernel_spmd

B, NQ, NK, D, H = 2, 2048, 2048, 1024, 16
DH = D // H  # 64
N_CORES = 8
HPC = H // (N_CORES // B)  # heads per core = 4
JW = HPC * DH  # per-core projection width = 256
NKT = NK // 128  # 16 nk tiles
NCH = 4  # nq chunks
CHW = NQ // NCH  # 512
DT = 8  # contraction d-tiles

f32 = mybir.dt.float32
bf16 = mybir.dt.bfloat16
u8 = mybir.dt.uint8


def _build():
    nc = bacc.Bacc(
        "TRN2", target_bir_lowering=False, debug=False, num_devices=N_CORES
    )

    # x tensors in partition-major chunk layout: X[p, ch, d, n] =
    # x[ch*CHW + n, d*128 + p] -- one contiguous 8KB run per partition
    # per chunk, so a chunk is a single 128-descriptor dma_start.
    qTd = nc.dram_tensor("qT", [128, NCH, DT, CHW], bf16, kind="ExternalInput")
    kTd = nc.dram_tensor("kT", [128, NCH, DT, CHW], bf16, kind="ExternalInput")
    vTd = nc.dram_tensor("vT", [128, NCH, DT, CHW], bf16, kind="ExternalInput")
    # mask: M[p, t, n] = mask[n, t*128 + p]
    maskd = nc.dram_tensor("maskT", [128, NKT, NQ], u8, kind="ExternalInput")
    # weights: W[p, d, j] = w[d*128 + p, j]
    wqd = nc.dram_tensor("wqT", [128, DT, JW], bf16, kind="ExternalInput")
    wkd = nc.dram_tensor("wkT", [128, DT, JW], bf16, kind="ExternalInput")
    wvd = nc.dram_tensor("wvT", [128, DT, JW], bf16, kind="ExternalInput")
    bqd = nc.dram_tensor("bq", [128, 2], f32, kind="ExternalInput")
    bkd = nc.dram_tensor("bk", [128, 2], f32, kind="ExternalInput")
    # bv2[64p + 0, 0:64] = bv[128p + dh], bv2[64p + 32, 64:128] = bv[...]
    bvd = nc.dram_tensor("bv2", [128, 128], f32, kind="ExternalInput")
    # ones2[64p, 0:64] = 1, ones2[64p + 32, 64:128] = 1
    onesd = nc.dram_tensor("ones2", [128, 128], f32, kind="ExternalInput")
    onespd = nc.dram_tensor("onesp", [128, 32], bf16, kind="ExternalInput")
    o = nc.dram_tensor("o", [2 * 128, NQ], bf16, kind="ExternalOutput")

    with tile.TileContext(nc) as tc:
        with (
            tc.tile_pool(name="consts", bufs=1) as consts,
            tc.tile_pool(name="kst", bufs=2) as kst,
            tc.tile_pool(name="qst", bufs=2) as qst,
            tc.tile_pool(name="vst", bufs=4) as vst,
            tc.tile_pool(name="m8pool", bufs=8) as m8pool,
            tc.tile_pool(name="mbpool", bufs=7) as mbpool,
            tc.tile_pool(name="projout", bufs=1) as projout,
            tc.tile_pool(name="ppool", bufs=16) as ppool,
            tc.tile_pool(name="rspool", bufs=2) as rspool,
            tc.tile_pool(name="outsb", bufs=4) as outsb,
            tc.tile_pool(name="sps", bufs=2, space="PSUM") as sps,
            tc.tile_pool(name="pvps", bufs=2, space="PSUM") as pvps,
            tc.tile_pool(name="rsps", bufs=2, space="PSUM") as rsps,
        ):
            def dma_w(name, dram):
                t = consts.tile([128, DT, JW], bf16, tag=f"w{name}", name="w")
                nc.sync.dma_start(t, dram[:])
                return t

            def dma_x(src, ch, pool, tag, split=False):
                x = pool.tile([128, DT, CHW], bf16, tag=tag, name=tag)
                if split:
                    # two halves so the d 0-3 matmuls can start earlier
                    nc.sync.dma_start(x[:, 0:4], src[:, ch, 0:4])
                    nc.sync.dma_start(x[:, 4:8], src[:, ch, 4:8])
                else:
                    nc.sync.dma_start(x, src[:, ch])
                return x

            def dma_m(g):
                """Mask tiles 2g, 2g+1. On the sync queue, placed in
                consumption order: the DMA engines drain mostly FIFO, so a
                parallel-queue mask would overtake the critical k/q path."""
                mt8 = m8pool.tile([128, 2, NQ], u8, tag="m8", name="m8")
                nc.sync.dma_start(mt8, maskd[:, 2 * g : 2 * g + 2, :])
                return mt8

            # ---- input DMAs, emitted in consumption order ----
            bq_sb = consts.tile([128, 2], f32, tag="bq")
            nc.sync.dma_start(bq_sb, bqd[:])
            bk_sb = consts.tile([128, 2], f32, tag="bk")
            nc.sync.dma_start(bk_sb, bkd[:])
            onesp_sb = consts.tile([128, 32], bf16, tag="onesp")
            nc.sync.dma_start(onesp_sb, onespd[:])
            w_k = dma_w("k", wkd)
            k_x = {0: dma_x(kTd, 0, kst, "kx", split=True)}
            w_q = dma_w("q", wqd)
            q_x = {0: dma_x(qTd, 0, qst, "qx", split=True)}
            m8 = [dma_m(0)]
            k_x[1] = dma_x(kTd, 1, kst, "kx")
            m8.append(dma_m(1))
            w_v = dma_w("v", wvd)
            v_x = {0: dma_x(vTd, 0, vst, "vx")}
            m8.append(dma_m(2))
            v_x[1] = dma_x(vTd, 1, vst, "vx")
            k_x[2] = dma_x(kTd, 2, kst, "kx")
            m8.append(dma_m(3))
            v_x[2] = dma_x(vTd, 2, vst, "vx")
            m8.append(dma_m(4))
            m8.append(dma_m(5))
            k_x[3] = dma_x(kTd, 3, kst, "kx")
            v_x[3] = dma_x(vTd, 3, vst, "vx")
            m8.append(dma_m(6))
            m8.append(dma_m(7))
            bv_sb = consts.tile([128, 128], f32, tag="bv")
            nc.sync.dma_start(bv_sb, bvd[:])
            ones_sb = consts.tile([128, 128], f32, tag="ones")
            nc.sync.dma_start(ones_sb, onesd[:])
            q_x[1] = dma_x(qTd, 1, qst, "qx")
            q_x[2] = dma_x(qTd, 2, qst, "qx")
            q_x[3] = dma_x(qTd, 3, qst, "qx")

            # ---- projection outputs (split per chunk for clean deps) ----
            ktTs = [
                projout.tile([128, 2, CHW], bf16, tag=f"ktT{c}", name="ktT")
                for c in range(NCH)
            ]
            qtTs = [
                projout.tile([128, 2, CHW], bf16, tag=f"qtT{c}", name="qtT")
                for c in range(NCH)
            ]
            # vts[g][:, a, :] = vt for nk-tile 2g+a
            vts = [
                projout.tile([128, 2, JW], bf16, tag=f"vt{g}", name="vt")
                for g in range(NKT // 2)
            ]

            def proj_qk_sps(w, xs, dst, bias):
                """q/k projection chunk through one 2-bank sps tile:
                m0 -> cols 0:CHW, m1 -> cols CHW:2CHW."""
                ps = sps.tile([128, 2 * CHW], f32, tag="s", name="pps")
                for d in range(DT):
                    for m in range(2):
                        nc.tensor.matmul(
                            ps[:, m * CHW : (m + 1) * CHW],
                            w[:, d, m * 128 : (m + 1) * 128],
                            xs[:, d],
                            start=(d == 0),
                            stop=(d == DT - 1),
                        )
                for m in range(2):
                    nc.vector.tensor_scalar_add(
                        dst[:, m, :],
                        ps[:, m * CHW : (m + 1) * CHW],
                        bias[:, m : m + 1],
                    )

            def proj_qk_m(w, xs, dst, bias, m, pool, nm):
                """One m-half of a q/k projection chunk through a single
                [128, CHW] psum tile from `pool`."""
                ps = pool.tile([128, CHW], f32, tag=nm, name="pps")
                for d in range(DT):
                    nc.tensor.matmul(
                        ps,
                        w[:, d, m * 128 : (m + 1) * 128],
                        xs[:, d],
                        start=(d == 0),
                        stop=(d == DT - 1),
                    )
                nc.vector.tensor_scalar_add(
                    dst[:, m, :], ps, bias[:, m : m + 1]
                )

            def vproj_pair(g):
                """vt for nk-tiles 2g, 2g+1 through the rsps utility slot."""
                ps = rsps.tile([128, CHW], f32, tag="rspst", name="vps")
                ps2 = ps.rearrange("p (a j) -> p a j", a=2)
                for d in range(DT):
                    for a in range(2):
                        t = 2 * g + a
                        ch, nn_ = divmod(t, 4)
                        # start=True clears the WHOLE bank (all 512 cols) in
                        # the written partitions, so only the very first
                        # matmul of the packed pair may set it; the a=1
                        # group's first matmul overwrites where has_written
                        # is clear (bank-wide clear reset its bits too).
                        nc.tensor.matmul(
                            ps2[:, a],
                            v_x[ch][:, d, nn_ * 128 : (nn_ + 1) * 128],
                            w_v[:, d, :],
                            start=(d == 0 and a == 0),
                            stop=(d == DT - 1),
                        )
                nc.vector.tensor_copy(vts[g][:], ps2)

            # ---- attention ----
            def scores_group(pair, t, ch, p_tiles):
                sp = sps.tile([128, 2 * CHW], f32, tag="s", name="sp")
                kc, tt = ktTs[t // 4], t % 4
                for hh in range(2):
                    nc.tensor.matmul(
                        sp[:, hh * CHW : (hh + 1) * CHW],
                        kc[
                            64 * hh : 64 * (hh + 1),
                            pair,
                            tt * 128 : (tt + 1) * 128,
                        ],
                        qtTs[ch][64 * hh : 64 * (hh + 1), pair, :],
                        start=True,
                        stop=True,
                    )
                p = ppool.tile([128, 2 * CHW], bf16, tag="p", name="p")
                nc.scalar.activation(
                    out=p,
                    in_=sp,
                    func=mybir.ActivationFunctionType.Exp,
                    scale=1.0 / 32.0,
                )
                if pair == 0:
                    mb = mbpool.tile([128, CHW], bf16, tag="mb", name="mb")
                    # SWDGE cast DMA u8 -> bf16 (frees GpSimd compute)
                    nc.gpsimd.dma_start(
                        mb, m8[t // 2][:, t % 2, ch * CHW : (ch + 1) * CHW]
                    )
                    p_tiles[("mb", t)] = mb
                else:
                    mb = p_tiles[("mb", t)]
                p3 = p.rearrange("p (h c) -> p h c", h=2)
                nc.vector.tensor_mul(
                    p3,
                    p3,
                    mb.rearrange("p (a c) -> p a c", a=1).to_broadcast(
                        (128, 2, CHW)
                    ),
                )
                p_tiles[(pair, t)] = p

            def pv_t(t, p_tiles, pv_ps, rs_ps):
                st, sp_ = t == 0, t == NKT - 1
                g, a = divmod(t, 2)
                for pair in range(2):
                    p = p_tiles[(pair, t)]
                    for hh in range(2):
                        nc.tensor.matmul(
                            pv_ps[pair][64 * hh : 64 * (hh + 1), :],
                            vts[g][
                                :,
                                a,
                                128 * pair + 64 * hh : 128 * pair
                                + 64 * (hh + 1),
                            ],
                            p[:, hh * CHW : (hh + 1) * CHW],
                            start=st,
                            stop=sp_,
                            tile_position=(0, 64 * hh),
                        )
                for pair in range(2):
                    p = p_tiles[(pair, t)]
                    for hh in range(2):
                        hg = 2 * pair + hh
                        nc.tensor.matmul(
                            rs_ps[32 * hg : 32 * hg + 32, :],
                            onesp_sb[:, 0:32],
                            p[:, hh * CHW : (hh + 1) * CHW],
                            start=st,
                            stop=sp_,
                            tile_position=(0, 32 * hg),
                        )

            def chunk_tail(cs, pv_ps, rs_ps):
                # rowsum -> +eps -> reciprocal (all 128 rows valid: the M=32
                # rowsum matmuls wrote 32 identical rows per head). The
                # reciprocal runs on DVE so it never blocks the ACT stream.
                rs_sb = rspool.tile([128, CHW], f32, tag="rssb", name="rssb")
                nc.vector.tensor_scalar_add(rs_sb, rs_ps, 1e-6)
                rc_sb = rspool.tile([128, CHW], f32, tag="rcsb", name="rcsb")
                nc.vector.reciprocal_approx_fast(out=rc_sb, in_=rs_sb)
                for pair in range(2):
                    # pv += bv (x) rowsum   (rank-1 via K=64, rows 0 and 32)
                    nc.tensor.matmul(
                        pv_ps[pair],
                        bv_sb[64 * pair : 64 * (pair + 1), :],
                        rs_sb[64 * pair : 64 * (pair + 1), :],
                        start=False,
                        stop=True,
                    )
                    # rb = broadcast of 1/(rs+eps) to the pair's 128 rows
                    rb = rsps.tile([128, CHW], f32, tag="rspst", name="rb")
                    nc.tensor.matmul(
                        rb,
                        ones_sb[64 * pair : 64 * (pair + 1), :],
                        rc_sb[64 * pair : 64 * (pair + 1), :],
                        start=True,
                        stop=True,
                    )
                    rb_sb = outsb.tile([128, CHW], f32, tag="rbsb", name="rbsb")
                    nc.vector.tensor_copy(rb_sb, rb)
                    osb = outsb.tile([128, CHW], bf16, tag="o", name="osb")
                    nc.vector.tensor_mul(osb, pv_ps[pair], rb_sb)
                    nc.gpsimd.dma_start(
                        o[128 * pair : 128 * (pair + 1), cs], osb
                    )

            def new_pv_tiles():
                pv_ps = [
                    pvps.tile([128, CHW], f32, tag="pvpst", name=f"pv{i}")
                    for i in range(2)
                ]
                rs_ps = rsps.tile([128, CHW], f32, tag="rspst", name="rsps_t")
                return pv_ps, rs_ps

            # ---- pre-phase: k-proj chunk0 + q-proj chunk0 ----
            proj_qk_sps(w_k, k_x[0], ktTs[0], bk_sb)
            proj_qk_sps(w_q, q_x[0], qtTs[0], bq_sb)

            # chunk 0 utility-phase schedule (kc1 through the pvps slots
            # before PV claims them; everything else through the rsps
            # rotating slot). Keyed by iteration; placed so each phase's
            # inputs (DMA) land just before and its output is ready just
            # before its first consumer. Emitted AFTER that iteration's
            # scores so the exps fire first.
            util0 = {
                2: [
                    lambda: proj_qk_m(w_k, k_x[1], ktTs[1], bk_sb, 0, pvps, "pvpst"),
                    lambda: proj_qk_m(w_k, k_x[1], ktTs[1], bk_sb, 1, pvps, "pvpst"),
                ],
                3: [lambda: vproj_pair(0)],
                4: [lambda: vproj_pair(1)],
                5: [lambda: vproj_pair(2)],
                6: [lambda: proj_qk_m(w_k, k_x[2], ktTs[2], bk_sb, 0, rsps, "rspst")],
                7: [lambda: proj_qk_m(w_k, k_x[2], ktTs[2], bk_sb, 1, rsps, "rspst")],
                8: [lambda: vproj_pair(3)],
                9: [lambda: vproj_pair(4)],
                10: [lambda: proj_qk_m(w_k, k_x[3], ktTs[3], bk_sb, 0, rsps, "rspst")],
                11: [lambda: proj_qk_m(w_k, k_x[3], ktTs[3], bk_sb, 1, rsps, "rspst")],
                12: [lambda: vproj_pair(5)],
                13: [lambda: vproj_pair(6)],
                14: [lambda: vproj_pair(7)],
            }

            # All chunks fully interleaved. The last LAG PV steps + tail of
            # chunk c are carried into chunk c+1's first iterations so the
            # ACT stream never waits for a PE drain at a chunk boundary.
            # Chunk c+1's q-projection runs mid-chunk-c through the rsps
            # slot (free after the carried tail releases it).
            LAG = 3
            pending = []
            for ch in range(NCH):
                cs = slice(ch * CHW, (ch + 1) * CHW)
                p_tiles = {}
                pv_ps, rs_ps = None, None
                for t in range(NKT):
                    if (ch == 0 and t == 3) or (ch > 0 and t == 0):
                        pv_ps, rs_ps = new_pv_tiles()
                    for pair in range(2):
                        scores_group(pair, t, ch, p_tiles)
                    if ch == 0 and t in util0:
                        for fn in util0[t]:
                            fn()
                    if pending:
                        pending.pop(0)()
                    if t >= LAG:
                        pv_t(t - LAG, p_tiles, pv_ps, rs_ps)
                    if ch + 1 < NCH and t in ((14, 15) if ch == 0 else (12, 14)):
                        m = 0 if t < 14 or (ch == 0 and t == 14) else 1
                        proj_qk_m(
                            w_q, q_x[ch + 1], qtTs[ch + 1], bq_sb, m,
                            rsps, "rspst",
                        )

                def _carry(tt, p_tiles=p_tiles, pv_ps=pv_ps, rs_ps=rs_ps):
                    pv_t(tt, p_tiles, pv_ps, rs_ps)

                def _tail(cs=cs, pv_ps=pv_ps, rs_ps=rs_ps):
                    chunk_tail(cs, pv_ps, rs_ps)

                pending = [
                    lambda f=_carry: f(NKT - 3),
                    lambda f=_carry: f(NKT - 2),
                    lambda f=_carry, g=_tail: (f(NKT - 1), g()),
                ]
            for fn in pending:
                fn()

    nc.compile()
    return nc


_NC = None


def _get_nc():
    global _NC
    if _NC is None:
        _NC = _build()
    return _NC


def _shard(inputs):
    import ml_dtypes

    bfl = ml_dtypes.bfloat16
    q, k, v = inputs["q"], inputs["k"], inputs["v"]
    mask = inputs["mask"]
    Wq, bq, Wk, bk, Wv, bv = (
        inputs[n] for n in ("Wq", "bq", "Wk", "bk", "Wv", "bv")
    )

    def xfmt(x):
        # [N, D] -> [128, NCH, DT, CHW]: X[p, ch, d, n] = x[ch*CHW+n, 128d+p]
        return np.ascontiguousarray(
            np.asarray(x)
            .reshape(NCH, CHW, DT, 128)
            .transpose(3, 0, 2, 1)
            .astype(bfl)
        )

    def wfmt(w, j0):
        # [D, D] -> [128, DT, JW]: W[p, d, j] = w[j0+j, 128d+p]
        return np.ascontiguousarray(
            np.asarray(w)[j0 : j0 + JW, :].T.reshape(DT, 128, JW)
            .transpose(1, 0, 2)
            .astype(bfl)
        )

    qX = [xfmt(q[b]) for b in range(B)]
    kX = [xfmt(k[b]) for b in range(B)]
    vX = [xfmt(v[b]) for b in range(B)]
    # mask [NQ, NK] bool -> [128, NKT, NQ] u8: M[p, t, n] = mask[n, 128t+p]
    mX = [
        np.ascontiguousarray(
            np.asarray(mask[b]).T.reshape(NKT, 128, NQ).transpose(1, 0, 2)
        ).view(np.uint8)
        for b in range(B)
    ]
    ones2 = np.zeros((128, 128), np.float32)
    for p in range(2):
        ones2[64 * p, 0:64] = 1.0
        ones2[64 * p + 32, 64:128] = 1.0
    onesp = np.ones((128, 32), bfl)
    in_maps = []
    for c in range(N_CORES):
        b, jg = divmod(c, N_CORES // B)
        j0 = jg * JW
        bvs = np.asarray(bv, np.float32)[j0 : j0 + JW]
        bv2 = np.zeros((128, 128), np.float32)
        for p in range(2):
            bv2[64 * p, 0:64] = bvs[128 * p : 128 * p + 64]
            bv2[64 * p + 32, 64:128] = bvs[128 * p + 64 : 128 * p + 128]
        in_maps.append(
            {
                "qT": qX[b],
                "kT": kX[b],
                "vT": vX[b],
                "maskT": mX[b],
                "wqT": wfmt(Wq, j0),
                "wkT": wfmt(Wk, j0),
                "wvT": wfmt(Wv, j0),
                "bq": np.ascontiguousarray(
                    np.asarray(bq, np.float32)[j0 : j0 + JW].reshape(2, 128).T
                ),
                "bk": np.ascontiguousarray(
                    np.asarray(bk, np.float32)[j0 : j0 + JW].reshape(2, 128).T
                ),
                "bv2": bv2,
                "ones2": ones2,
                "onesp": onesp,
            }
        )
    return in_maps


LAST_RESULT = None


def kernel(**inputs) -> np.ndarray:
    global LAST_RESULT
    nc = _get_nc()
    in_maps = _shard(inputs)
    trace = bool(int(os.environ.get("KTRACE", "0")))
    res = run_bass_kernel_spmd(
        nc,
        in_maps,
        core_ids=list(range(N_CORES)),
        trace=trace,
        trace_cores=[0] if trace else None,
    )
    LAST_RESULT = res
    out = np.empty((B, NQ, D), np.float32)
    for c in range(N_CORES):
        b, jg = divmod(c, N_CORES // B)
        j0 = jg * JW
        oc = res.results[c]["o"].astype(np.float32)  # [256, NQ] pair-major
        out[b, :, j0 : j0 + JW] = (
            oc.reshape(2, 2, DH, NQ).transpose(3, 0, 1, 2).reshape(NQ, JW)
        )
    return out


if __name__ == "__main__":
    if os.environ.get("KBUILD_ONLY"):
        import tempfile

        from concourse.bass_utils import compile_bass_kernel

        nc = _build()
        with tempfile.TemporaryDirectory() as td:
            compile_bass_kernel(nc, td)
        print("BUILD+COMPILE OK")
